# revision 21
# baseline (speedup 1.0000x reference)
"""PSRoIPool (position-sensitive ROI pooling) for Trainium2, 8 NeuronCores.

Problem (hardcoded):
  features [4, 392, 128, 128] f32, rois [512, 5] f32 (batch, x1, y1, x2, y2)
  out [512, 8, 7, 7] f32;  C = C_out(8) * 7 * 7;  spatial_scale = 1/16.

Sharding: by output channel c_out (8 cores). Core k owns feature channels
[49k, 49k+49) of every image (1/8 of the features, read exactly once as
bf16) and computes out[:, k, :, :] for ALL 512 rois.

Algorithm (per core):
  - rois stable-sorted by batch -> exactly 4 windows of 128 rois. A window
    spanning a batch boundary is computed as 2 PSUM-accumulated matmul
    passes whose H-masks are zeroed outside their roi range (host-side).
  - stage 1 (PE, bf16): u[r, pw, w] = sum_h Mh[h, r; ph] * F[h, c_s, w]
    per (window, ph); H-mask is the stationary operand.
  - stage 2 (DVE, one fused pass): custom DVE op MASKED_PREFIX_ANT computes
    P[r, t] = prefix-sum of u[r, t] * Mw[r, t] over the flat (pw, w) axis.
    Bin sums drop out as strided differences at the 128-column boundaries:
      bins[r, ph, 0]    = P[r, 127]
      bins[r, ph, pw>0] = P[r, 128(pw+1)-1] - P[r, 128 pw - 1]
  - out = bins * recip_area (recip precomputed on host, 0 for empty bins).

All masks + reciprocal areas are precomputed on the host and DMAed (they
depend only on the rois, are identical on all 8 cores, and cost ~2.7 MB
vs 6.4 MB of bf16 features).
"""

import numpy as np
from contextlib import ExitStack

try:
    import ml_dtypes
    _BF16 = ml_dtypes.bfloat16
except ImportError:  # pragma: no cover
    import jax.numpy as _jnp
    _BF16 = _jnp.bfloat16

N_IMG, C_FULL, H, W = 4, 392, 128, 128
R = 512
P = 7  # OUT_SIZE == GROUP
C_OUT = 8
C_SLAB = P * P  # 49 channels per core
SCALE = np.float32(0.0625)
NCORES = 8
RW = 128  # rois per window
NWIN = R // RW

OP_NAME = "MASKED_PREFIX_ANT"


# ------------------------------------------------------------- custom DVE op
def _get_custom_op():
    """out[p, k] = sum_{j<=k} in0[p, j] * in1[p, j] (fp32 internal state).

    Registered by appending to concourse.dve_ops.OPS (the per-NEFF DVE
    table is generated from OPS by bass_utils.dve_table_for_ops)."""
    import concourse.dve_ops as D

    for op in D.OPS:
        if op.name == OP_NAME:
            return op

    from concourse.dve_spec import Spec, scan, AluOp, Src0, Src1, lower, _has_src1
    from concourse.dve_uop import DveOpSpec

    def ref(in0, in1, s0, s1, imm2):
        p = in0.astype(np.float32) * in1.astype(np.float32)
        return np.add.accumulate(p, axis=-1).astype(np.float32)

    spec = Spec(body=scan(AluOp.ADD, Src0 * Src1), reference=ref)
    row = max(D._SUB_OPCODE_FOR_NAME.values()) + 1
    assert row < 0x20
    shas = {}
    for ver in ("v3", "v4"):
        s = DveOpSpec(name=OP_NAME, opcode=row,
                      uops=lower(spec, ver=ver), rd1_en=_has_src1(spec))
        shas[ver] = s.sha(ver)
    op = D.DveOp(OP_NAME, spec, subdim=False, uops_sha=shas)
    D.OPS.append(op)
    D.CUSTOM_DVE_SPECS[op.name] = op.spec
    D._SUB_OPCODE_FOR_NAME[op.name] = row
    return op


# ---------------------------------------------------------------- host math
def _bounds(rois_sorted: np.ndarray):
    """Bit-exact f32 mirror of the reference coordinate transform."""
    r = rois_sorted.astype(np.float32)
    one = np.float32(1.0)
    rsw = np.round(r[:, 1]) * SCALE
    rsh = np.round(r[:, 2]) * SCALE
    rew = (np.round(r[:, 3]) + one) * SCALE
    reh = (np.round(r[:, 4]) + one) * SCALE
    roi_w = np.maximum(rew - rsw, np.float32(0.1))
    roi_h = np.maximum(reh - rsh, np.float32(0.1))
    bin_w = (roi_w / np.float32(P)).astype(np.float32)
    bin_h = (roi_h / np.float32(P)).astype(np.float32)
    p = np.arange(P, dtype=np.float32)
    hs = np.clip(np.floor(p[None, :] * bin_h[:, None] + rsh[:, None]), 0, H)
    he = np.clip(np.ceil((p[None, :] + one) * bin_h[:, None] + rsh[:, None]), 0, H)
    ws = np.clip(np.floor(p[None, :] * bin_w[:, None] + rsw[:, None]), 0, W)
    we = np.clip(np.ceil((p[None, :] + one) * bin_w[:, None] + rsw[:, None]), 0, W)
    return hs, he, ws, we  # [R, P] f32 (integer-valued)


def _plan(batch: np.ndarray):
    """Pack rois into NWIN windows of RW, minimising matmul streams: pull
    whole-window pure-batch groups first, then chunk the remainders.
    Returns (order [R], pieces: per-window ((b, c0, c1), ...))."""
    groups = [np.nonzero(batch == b)[0] for b in range(N_IMG)]
    pure, rem = [], []
    for idx in groups:
        n_pure = len(idx) // RW
        for i in range(n_pure):
            pure.append(idx[i * RW:(i + 1) * RW])
        rem.append(idx[n_pure * RW:])
    rem = np.concatenate([r for r in rem if len(r)]) if any(
        len(r) for r in rem) else np.empty(0, np.int64)
    order = np.concatenate(pure + ([rem] if len(rem) else []))
    assert len(order) == R
    sb = batch[order]
    pieces = []
    for w in range(NWIN):
        seg = sb[w * RW:(w + 1) * RW]
        ps = []
        start = 0
        for i in range(1, RW + 1):
            if i == RW or seg[i] != seg[start]:
                ps.append((int(seg[start]), start, i))
                start = i
        pieces.append(tuple(ps))
    return order, tuple(pieces)


def _prep(rois: np.ndarray):
    batch = rois[:, 0].astype(np.int32)
    order, pieces = _plan(batch)
    rs = rois[order]
    hs, he, ws, we = _bounds(rs)

    harange = np.arange(H, dtype=np.float32)
    warange = np.arange(W, dtype=np.float32)

    # hmask streams: one per (window, piece): [h, ph, r] zeroed outside piece
    hm_list = []
    for w, ps in enumerate(pieces):
        sl = slice(w * RW, (w + 1) * RW)
        hsw, hew = hs[sl], he[sl]  # [RW, P]
        m = ((harange[:, None, None] >= hsw.T[None, :, :])
             & (harange[:, None, None] < hew.T[None, :, :]))  # [H, P, RW]
        for (b, c0, c1) in ps:
            mm = np.zeros((H, P, RW), dtype=np.float32)
            mm[:, :, c0:c1] = m[:, :, c0:c1]
            hm_list.append(mm.reshape(H, P * RW))
    hmask = np.asarray(np.stack(hm_list), dtype=_BF16)  # [NS, 128, 896]

    # W-interval mask per window: [r, pw, w]
    mw = ((warange[None, None, :] >= ws[:, :, None])
          & (warange[None, None, :] < we[:, :, None])).astype(np.float32)
    mww = np.asarray(mw.reshape(NWIN, RW, P * W), dtype=_BF16)

    # open-ended masks for the gpsimd scan path: (w >= max(t, 1)) so the
    # (u + state) * m recurrence always resets at each 128-row start; the
    # dropped w=0 term is added back as zz * u[:, :, 0]
    ma = (warange[None, None, :] >= np.maximum(ws, 1.0)[:, :, None])
    mb = (warange[None, None, :] >= np.maximum(we, 1.0)[:, :, None])
    mao = np.asarray(ma.reshape(NWIN, RW, P * W).astype(np.float32), dtype=_BF16)
    mbo = np.asarray(mb.reshape(NWIN, RW, P * W).astype(np.float32), dtype=_BF16)
    zz = ((ws == 0).astype(np.float32) - (we == 0).astype(np.float32))
    zz = zz.reshape(NWIN, RW, P).astype(np.float32)

    # reciprocal area, 0 where empty  [NWIN, RW, 49] (cs = ph*7+pw)
    ah = he - hs  # [R, P]
    aw = we - ws
    area = ah[:, :, None] * aw[:, None, :]  # [R, ph, pw]
    recip = np.where(area > 0, np.float32(1.0) / np.maximum(area, 1.0),
                     np.float32(0.0)).astype(np.float32)
    recip = recip.reshape(NWIN, RW, C_SLAB)

    return order, pieces, hmask, mww, recip, mao, mbo, zz


# ---------------------------------------------------------------- device IR
def build_program(pieces, repeat=1, dma_once=False, gp_phs=()):
    import concourse.bass as bass
    import concourse.tile as tile
    from concourse import bacc, mybir

    f32 = mybir.dt.float32
    bf16 = mybir.dt.bfloat16
    Alu = mybir.AluOpType
    op = _get_custom_op()

    nstream = sum(len(ps) for ps in pieces)
    CW = C_SLAB * W  # 6272

    nc = bacc.Bacc("TRN2", target_bir_lowering=False, debug=False,
                   num_devices=NCORES)

    # [N, H, CS, W] so each partition line is CS*W contiguous bytes
    fslab = nc.dram_tensor("fslab", [N_IMG, H, C_SLAB, W], bf16,
                           kind="ExternalInput").ap()
    hmask = nc.dram_tensor("hmask", [nstream, H, P * RW], bf16,
                           kind="ExternalInput").ap()
    mww = nc.dram_tensor("mww", [NWIN, RW, P * W], bf16,
                         kind="ExternalInput").ap()
    recip = nc.dram_tensor("recip", [NWIN, RW, C_SLAB], f32,
                           kind="ExternalInput").ap()
    if gp_phs:
        mao = nc.dram_tensor("mao", [NWIN, RW, P * W], bf16,
                             kind="ExternalInput").ap()
        mbo = nc.dram_tensor("mbo", [NWIN, RW, P * W], bf16,
                             kind="ExternalInput").ap()
        zz = nc.dram_tensor("zz", [NWIN, RW, P], f32,
                            kind="ExternalInput").ap()
    out = nc.dram_tensor("out", [R, C_SLAB], f32, kind="ExternalOutput").ap()

    with tile.TileContext(nc) as tc, ExitStack() as ctx:
        fpool = ctx.enter_context(tc.tile_pool(name="fs", bufs=2))
        mpool = ctx.enter_context(tc.tile_pool(name="masks", bufs=2))
        spool = ctx.enter_context(tc.tile_pool(name="scratch", bufs=3))
        bpool = ctx.enter_context(tc.tile_pool(name="bins", bufs=2))
        opool = ctx.enter_context(tc.tile_pool(name="outw", bufs=2))
        psum = ctx.enter_context(tc.tile_pool(name="ps", bufs=3, space="PSUM"))

        for _rep in range(repeat):
            if _rep == 0 or not dma_once:
                fs = []
                for b in range(N_IMG):
                    fsb = fpool.tile([128, C_SLAB, W], bf16, tag=f"fs{b}")
                    src = bass.AP(tensor=fslab.tensor, offset=b * H * CW,
                                  ap=[[CW, H], [W, C_SLAB], [1, W]])
                    nc.sync.dma_start(out=fsb[:], in_=src)
                    fs.append(fsb)

            sidx = 0
            for win in range(NWIN):
                ps = pieces[win]
                hm = []
                for _ in ps:
                    hmt = mpool.tile([128, P * RW], bf16, tag=f"hm{sidx}")
                    nc.sync.dma_start(
                        out=hmt[:],
                        in_=bass.AP(tensor=hmask.tensor,
                                    offset=sidx * H * P * RW,
                                    ap=[[P * RW, H], [1, P * RW]]))
                    hm.append(hmt)
                    sidx += 1
                mwt = mpool.tile([128, P * W], bf16, tag="mw")
                nc.sync.dma_start(
                    out=mwt[:],
                    in_=bass.AP(tensor=mww.tensor, offset=win * RW * P * W,
                                ap=[[P * W, RW], [1, P * W]]))
                rct = mpool.tile([128, C_SLAB], f32, tag="rc")
                nc.sync.dma_start(
                    out=rct[:],
                    in_=bass.AP(tensor=recip.tensor, offset=win * RW * C_SLAB,
                                ap=[[C_SLAB, RW], [1, C_SLAB]]))
                if gp_phs:
                    mat = mpool.tile([128, P * W], bf16, tag="ma")
                    nc.sync.dma_start(
                        out=mat[:],
                        in_=bass.AP(tensor=mao.tensor, offset=win * RW * P * W,
                                    ap=[[P * W, RW], [1, P * W]]))
                    mbt = mpool.tile([128, P * W], bf16, tag="mb")
                    nc.sync.dma_start(
                        out=mbt[:],
                        in_=bass.AP(tensor=mbo.tensor, offset=win * RW * P * W,
                                    ap=[[P * W, RW], [1, P * W]]))
                    zzt = mpool.tile([128, P], f32, tag="zz")
                    nc.sync.dma_start(
                        out=zzt[:],
                        in_=bass.AP(tensor=zz.tensor, offset=win * RW * P,
                                    ap=[[P, RW], [1, P]]))

                bins = bpool.tile([128, C_SLAB], f32, tag="bins")

                for ph in range(P):
                    u = psum.tile([128, P, W], f32, tag="u")
                    for (n0, n1) in ((0, 4), (4, P)):
                        for i, (b, c0, c1) in enumerate(ps):
                            nc.tensor.matmul(
                                out=u[:, n0:n1, :],
                                lhsT=hm[i][:, ph * RW:(ph + 1) * RW],
                                rhs=fs[b][:, ph * P + n0:ph * P + n1, :],
                                start=(i == 0), stop=(i == len(ps) - 1))
                    cs0 = ph * P
                    if ph in gp_phs:
                        # gpsimd path: ACT copies u to SBUF (gpsimd can't read
                        # PSUM), then 2 open-mask scans; bins = suffix-sum
                        # differences + [t==0] * u0 correction
                        usb = spool.tile([128, P, W], bf16, tag="usb")
                        nc.scalar.copy(usb[:], u[:])
                        usb_flat = bass.AP(tensor=usb.tensor, offset=usb.offset,
                                           ap=[usb.ap[0], [1, P * W]])
                        sa = spool.tile([128, P * W], f32, tag="sa")
                        nc.gpsimd.tensor_tensor_scan(
                            out=sa[:], data0=usb_flat, data1=mat[:],
                            initial=0.0, op0=Alu.add, op1=Alu.mult)
                        sb_ = spool.tile([128, P * W], f32, tag="sb")
                        nc.gpsimd.tensor_tensor_scan(
                            out=sb_[:], data0=usb_flat, data1=mbt[:],
                            initial=0.0, op0=Alu.add, op1=Alu.mult)
                        ea = bass.AP(tensor=sa.tensor, offset=sa.offset + W - 1,
                                     ap=[sa.ap[0], [W, P]])
                        eb = bass.AP(tensor=sb_.tensor, offset=sb_.offset + W - 1,
                                     ap=[sb_.ap[0], [W, P]])
                        u0 = bass.AP(tensor=usb.tensor, offset=usb.offset,
                                     ap=[usb.ap[0], [W, P]])
                        raw = spool.tile([128, P], f32, tag="raw")
                        nc.any.tensor_tensor(out=raw[:], in0=ea, in1=eb,
                                             op=Alu.subtract)
                        fx = spool.tile([128, P], f32, tag="fx")
                        nc.any.tensor_tensor(out=fx[:], in0=zzt[:], in1=u0,
                                             op=Alu.mult)
                        nc.any.tensor_tensor(out=bins[:, cs0:cs0 + P],
                                             in0=raw[:], in1=fx[:],
                                             op=Alu.add)
                        continue
                    pre = spool.tile([128, P * W], f32, tag="pre")
                    nc.vector._custom_dve(op, out=pre[:], in0=u[:], in1=mwt[:])
                    nc.scalar.copy(bins[:, cs0:cs0 + 1], pre[:, W - 1:W])
                    hi = bass.AP(tensor=pre.tensor, offset=pre.offset + 2 * W - 1,
                                 ap=[pre.ap[0], [W, P - 1]])
                    lo = bass.AP(tensor=pre.tensor, offset=pre.offset + W - 1,
                                 ap=[pre.ap[0], [W, P - 1]])
                    nc.gpsimd.tensor_tensor(out=bins[:, cs0 + 1:cs0 + P],
                                            in0=hi, in1=lo, op=Alu.subtract)

                outw = opool.tile([128, C_SLAB], f32, tag="outw")
                nc.gpsimd.tensor_tensor(out=outw[:], in0=bins[:], in1=rct[:],
                                        op=Alu.mult)
                nc.sync.dma_start(out=out[win * RW:(win + 1) * RW, :],
                                  in_=outw[:])

    nc.compile()
    return nc


_PROG_CACHE = {}

# default stage-2 split: phs listed here run on gpsimd (2-scan), rest on DVE
GP_PHS = ()


def _get_program(pieces, repeat=1):
    key = (pieces, repeat, GP_PHS)
    if key not in _PROG_CACHE:
        _PROG_CACHE[key] = build_program(pieces, repeat=repeat, gp_phs=GP_PHS)
    return _PROG_CACHE[key]


# ---------------------------------------------------------------- entrypoint
def kernel(features: np.ndarray, rois: np.ndarray) -> np.ndarray:
    from concourse.bass_utils import run_bass_kernel_spmd

    features = np.asarray(features, dtype=np.float32)
    rois = np.asarray(rois, dtype=np.float32)

    order, pieces, hmask, mww, recip, mao, mbo, zz = _prep(rois)
    nc = _get_program(pieces)

    # [N, CS, H, W] -> [N, H, CS, W], bf16
    fbf = features.astype(_BF16)
    in_maps = []
    for k in range(NCORES):
        slab = fbf[:, k * C_SLAB:(k + 1) * C_SLAB].transpose(0, 2, 1, 3)
        m = {
            "fslab": np.ascontiguousarray(slab),
            "hmask": hmask,
            "mww": mww,
            "recip": recip,
        }
        if GP_PHS:
            m.update({"mao": mao, "mbo": mbo, "zz": zz})
        in_maps.append(m)

    res = run_bass_kernel_spmd(nc, in_maps, list(range(NCORES))).results

    result = np.empty((R, C_OUT, C_SLAB), dtype=np.float32)
    for k in range(NCORES):
        result[order, k, :] = res[k]["out"]
    return result.reshape(R, C_OUT, P, P)


# revision 28
# speedup vs baseline: 1.1932x; 1.1932x over previous
"""PSRoIPool (position-sensitive ROI pooling) for Trainium2, 8 NeuronCores.

Problem (hardcoded):
  features [4, 392, 128, 128] f32, rois [512, 5] f32 (batch, x1, y1, x2, y2)
  out [512, 8, 7, 7] f32;  C = C_out(8) * 7 * 7;  spatial_scale = 1/16.

Sharding: by output channel c_out (8 cores). Core k owns feature channels
[49k, 49k+49) of every image (1/8 of the features, read exactly once as
bf16) and computes out[:, k, :, :] for ALL 512 rois.

Algorithm (per core):
  - rois stable-sorted by batch -> exactly 4 windows of 128 rois. A window
    spanning a batch boundary is computed as 2 PSUM-accumulated matmul
    passes whose H-masks are zeroed outside their roi range (host-side).
  - stage 1 (PE, bf16): u[r, pw, w] = sum_h Mh[h, r; ph] * F[h, c_s, w]
    per (window, ph); H-mask is the stationary operand.
  - stage 2 (DVE, one fused pass): custom DVE op MASKED_PREFIX_ANT computes
    P[r, t] = prefix-sum of u[r, t] * Mw[r, t] over the flat (pw, w) axis.
    Bin sums drop out as strided differences at the 128-column boundaries:
      bins[r, ph, 0]    = P[r, 127]
      bins[r, ph, pw>0] = P[r, 128(pw+1)-1] - P[r, 128 pw - 1]
  - out = bins * recip_area (recip precomputed on host, 0 for empty bins).

All masks + reciprocal areas are precomputed on the host and DMAed (they
depend only on the rois, are identical on all 8 cores, and cost ~2.7 MB
vs 6.4 MB of bf16 features).
"""

import numpy as np
from contextlib import ExitStack

try:
    import ml_dtypes
    _BF16 = ml_dtypes.bfloat16
except ImportError:  # pragma: no cover
    import jax.numpy as _jnp
    _BF16 = _jnp.bfloat16

N_IMG, C_FULL, H, W = 4, 392, 128, 128
R = 512
P = 7  # OUT_SIZE == GROUP
C_OUT = 8
C_SLAB = P * P  # 49 channels per core
SCALE = np.float32(0.0625)
NCORES = 8
RW = 128  # rois per window
NWIN = R // RW

OP_NAME = "MASKED_PREFIX_ANT"


# ------------------------------------------------------------- custom DVE op
def _get_custom_op():
    """out[p, k] = sum_{j<=k} in0[p, j] * in1[p, j] (fp32 internal state).

    Registered by appending to concourse.dve_ops.OPS (the per-NEFF DVE
    table is generated from OPS by bass_utils.dve_table_for_ops)."""
    import concourse.dve_ops as D

    for op in D.OPS:
        if op.name == OP_NAME:
            return op

    from concourse.dve_spec import Spec, scan, AluOp, Src0, Src1, lower, _has_src1
    from concourse.dve_uop import DveOpSpec

    def ref(in0, in1, s0, s1, imm2):
        p = in0.astype(np.float32) * in1.astype(np.float32)
        return np.add.accumulate(p, axis=-1).astype(np.float32)

    spec = Spec(body=scan(AluOp.ADD, Src0 * Src1), reference=ref)
    row = max(D._SUB_OPCODE_FOR_NAME.values()) + 1
    assert row < 0x20
    shas = {}
    for ver in ("v3", "v4"):
        s = DveOpSpec(name=OP_NAME, opcode=row,
                      uops=lower(spec, ver=ver), rd1_en=_has_src1(spec))
        shas[ver] = s.sha(ver)
    op = D.DveOp(OP_NAME, spec, subdim=False, uops_sha=shas)
    D.OPS.append(op)
    D.CUSTOM_DVE_SPECS[op.name] = op.spec
    D._SUB_OPCODE_FOR_NAME[op.name] = row
    return op


# ---------------------------------------------------------------- host math
def _bounds(rois_sorted: np.ndarray):
    """Bit-exact f32 mirror of the reference coordinate transform."""
    r = rois_sorted.astype(np.float32)
    one = np.float32(1.0)
    rsw = np.round(r[:, 1]) * SCALE
    rsh = np.round(r[:, 2]) * SCALE
    rew = (np.round(r[:, 3]) + one) * SCALE
    reh = (np.round(r[:, 4]) + one) * SCALE
    roi_w = np.maximum(rew - rsw, np.float32(0.1))
    roi_h = np.maximum(reh - rsh, np.float32(0.1))
    bin_w = (roi_w / np.float32(P)).astype(np.float32)
    bin_h = (roi_h / np.float32(P)).astype(np.float32)
    p = np.arange(P, dtype=np.float32)
    hs = np.clip(np.floor(p[None, :] * bin_h[:, None] + rsh[:, None]), 0, H)
    he = np.clip(np.ceil((p[None, :] + one) * bin_h[:, None] + rsh[:, None]), 0, H)
    ws = np.clip(np.floor(p[None, :] * bin_w[:, None] + rsw[:, None]), 0, W)
    we = np.clip(np.ceil((p[None, :] + one) * bin_w[:, None] + rsw[:, None]), 0, W)
    return hs, he, ws, we  # [R, P] f32 (integer-valued)


def _plan(batch: np.ndarray):
    """Pack rois into NWIN windows of RW, minimising matmul streams: pull
    whole-window pure-batch groups first, then chunk the remainders.
    Returns (order [R], pieces: per-window ((b, c0, c1), ...))."""
    groups = [np.nonzero(batch == b)[0] for b in range(N_IMG)]
    pure, rem = [], []
    for idx in groups:
        n_pure = len(idx) // RW
        for i in range(n_pure):
            pure.append(idx[i * RW:(i + 1) * RW])
        rem.append(idx[n_pure * RW:])
    rem = np.concatenate([r for r in rem if len(r)]) if any(
        len(r) for r in rem) else np.empty(0, np.int64)
    order = np.concatenate(pure + ([rem] if len(rem) else []))
    assert len(order) == R
    sb = batch[order]
    pieces = []
    for w in range(NWIN):
        seg = sb[w * RW:(w + 1) * RW]
        ps = []
        start = 0
        for i in range(1, RW + 1):
            if i == RW or seg[i] != seg[start]:
                ps.append((int(seg[start]), start, i))
                start = i
        pieces.append(tuple(ps))
    return order, tuple(pieces)


def _prep(rois: np.ndarray):
    batch = rois[:, 0].astype(np.int32)
    order, pieces = _plan(batch)
    rs = rois[order]
    hs, he, ws, we = _bounds(rs)

    harange = np.arange(H, dtype=np.float32)
    warange = np.arange(W, dtype=np.float32)

    # hmask streams: one per (window, piece): [h, ph, r] zeroed outside piece
    hm_list = []
    for w, ps in enumerate(pieces):
        sl = slice(w * RW, (w + 1) * RW)
        hsw, hew = hs[sl], he[sl]  # [RW, P]
        m = ((harange[:, None, None] >= hsw.T[None, :, :])
             & (harange[:, None, None] < hew.T[None, :, :]))  # [H, P, RW]
        for (b, c0, c1) in ps:
            mm = np.zeros((H, P, RW), dtype=np.float32)
            mm[:, :, c0:c1] = m[:, :, c0:c1]
            hm_list.append(mm.reshape(H, P * RW))
    hmask = np.asarray(np.stack(hm_list), dtype=_BF16)  # [NS, 128, 896]

    # W-interval mask per window: [r, pw, w]
    mw = ((warange[None, None, :] >= ws[:, :, None])
          & (warange[None, None, :] < we[:, :, None])).astype(np.float32)
    mww = np.asarray(mw.reshape(NWIN, RW, P * W), dtype=_BF16)

    # open-ended masks for the gpsimd scan path: (w >= max(t, 1)) so the
    # (u + state) * m recurrence always resets at each 128-row start; the
    # dropped w=0 term is added back as zz * u[:, :, 0]
    ma = (warange[None, None, :] >= np.maximum(ws, 1.0)[:, :, None])
    mb = (warange[None, None, :] >= np.maximum(we, 1.0)[:, :, None])
    mao = np.asarray(ma.reshape(NWIN, RW, P * W).astype(np.float32), dtype=_BF16)
    mbo = np.asarray(mb.reshape(NWIN, RW, P * W).astype(np.float32), dtype=_BF16)
    zz = ((ws == 0).astype(np.float32) - (we == 0).astype(np.float32))
    zz = zz.reshape(NWIN, RW, P).astype(np.float32)

    # reciprocal area, 0 where empty  [NWIN, RW, 49] (cs = ph*7+pw)
    ah = he - hs  # [R, P]
    aw = we - ws
    area = ah[:, :, None] * aw[:, None, :]  # [R, ph, pw]
    recip = np.where(area > 0, np.float32(1.0) / np.maximum(area, 1.0),
                     np.float32(0.0)).astype(np.float32)
    recip = recip.reshape(NWIN, RW, C_SLAB)

    return order, pieces, hmask, mww, recip, mao, mbo, zz


# ---------------------------------------------------------------- device IR
def build_program(pieces, repeat=1, dma_once=False):
    import concourse.bass as bass
    import concourse.tile as tile
    from concourse import bacc, mybir

    f32 = mybir.dt.float32
    bf16 = mybir.dt.bfloat16
    Alu = mybir.AluOpType
    op = _get_custom_op()

    nstream = sum(len(ps) for ps in pieces)
    CW = C_SLAB * W  # 6272

    nc = bacc.Bacc("TRN2", target_bir_lowering=False, debug=False,
                   num_devices=NCORES)

    # [N, H, CS, W] so each partition line is CS*W contiguous bytes
    fslab = nc.dram_tensor("fslab", [N_IMG, H, C_SLAB, W], bf16,
                           kind="ExternalInput").ap()
    hmask = nc.dram_tensor("hmask", [nstream, H, P * RW], bf16,
                           kind="ExternalInput").ap()
    mww = nc.dram_tensor("mww", [NWIN, RW, P * W], bf16,
                         kind="ExternalInput").ap()
    recip = nc.dram_tensor("recip", [NWIN, RW, C_SLAB], f32,
                           kind="ExternalInput").ap()
    out = nc.dram_tensor("out", [R, C_SLAB], f32, kind="ExternalOutput").ap()

    with tile.TileContext(nc) as tc, ExitStack() as ctx:
        fpool = ctx.enter_context(tc.tile_pool(name="fs", bufs=2))
        mpool = ctx.enter_context(tc.tile_pool(name="masks", bufs=2))
        spool = ctx.enter_context(tc.tile_pool(name="scratch", bufs=3))
        bpool = ctx.enter_context(tc.tile_pool(name="bins", bufs=2))
        opool = ctx.enter_context(tc.tile_pool(name="outw", bufs=2))
        # u2 tiles are [128, 14, 128] f32 = 3.5 PSUM banks; 2 bufs = 7 of 8
        psum = ctx.enter_context(tc.tile_pool(name="ps", bufs=2, space="PSUM"))

        for _rep in range(repeat):
            if _rep == 0 or not dma_once:
                fs = []
                for b in range(N_IMG):
                    fsb = fpool.tile([128, C_SLAB, W], bf16, tag=f"fs{b}")
                    src = bass.AP(tensor=fslab.tensor, offset=b * H * CW,
                                  ap=[[CW, H], [W, C_SLAB], [1, W]])
                    nc.sync.dma_start(out=fsb[:], in_=src)
                    fs.append(fsb)

            sidx = 0
            for win in range(NWIN):
                ps = pieces[win]
                hm = []
                for _ in ps:
                    hmt = mpool.tile([128, P * RW], bf16, tag=f"hm{sidx}")
                    nc.sync.dma_start(
                        out=hmt[:],
                        in_=bass.AP(tensor=hmask.tensor,
                                    offset=sidx * H * P * RW,
                                    ap=[[P * RW, H], [1, P * RW]]))
                    hm.append(hmt)
                    sidx += 1
                mwt = mpool.tile([128, P * W], bf16, tag="mw")
                nc.sync.dma_start(
                    out=mwt[:],
                    in_=bass.AP(tensor=mww.tensor, offset=win * RW * P * W,
                                ap=[[P * W, RW], [1, P * W]]))
                rct = mpool.tile([128, C_SLAB], f32, tag="rc")
                nc.sync.dma_start(
                    out=rct[:],
                    in_=bass.AP(tensor=recip.tensor, offset=win * RW * C_SLAB,
                                ap=[[C_SLAB, RW], [1, C_SLAB]]))
                bins = bpool.tile([128, C_SLAB], f32, tag="bins")

                # ph groups: pairs (0,1),(2,3),(4,5) + single (6,). One fused
                # scan per group over the PSUM tile; matmul column-splits are
                # PSUM-bank-aligned AND split at the ph boundary (row 7):
                # rows 0-6 use ph a's H-mask, rows 7-13 ph b's.
                for grp in ((0, 1), (2, 3), (4, 5), (6,)):
                    pa = grp[0]
                    nrow = 7 * len(grp)
                    base = pa * P  # contiguous channels base+0 .. base+nrow-1
                    u = psum.tile([128, 2 * P, W], f32, tag="u2")
                    if len(grp) == 2:
                        splits = ((0, 4, grp[0]), (4, 7, grp[0]),
                                  (7, 8, grp[1]), (8, 12, grp[1]),
                                  (12, 14, grp[1]))
                    else:
                        splits = ((0, 4, grp[0]), (4, 7, grp[0]))
                    for (n0, n1, p_) in splits:
                        for i, (b, c0, c1) in enumerate(ps):
                            nc.tensor.matmul(
                                out=u[:, n0:n1, :],
                                lhsT=hm[i][:, p_ * RW:(p_ + 1) * RW],
                                rhs=fs[b][:, base + n0:base + n1, :],
                                start=(i == 0), stop=(i == len(ps) - 1))
                    pre = spool.tile([128, 2 * P * W], f32, tag="pre")
                    u_flat = bass.AP(tensor=u.tensor, offset=u.offset,
                                     ap=[u.ap[0], [1, nrow * W]])
                    if len(grp) == 2:
                        # mask streams twice (same [r, pw, w] mask per ph)
                        m_in = bass.AP(tensor=mwt.tensor, offset=mwt.offset,
                                       ap=[mwt.ap[0], [0, 2], [1, P * W]])
                    else:
                        m_in = mwt[:]
                    nc.vector._custom_dve(op, out=pre[:, 0:nrow * W],
                                          in0=u_flat, in1=m_in)
                    cs0 = base
                    nc.scalar.copy(bins[:, cs0:cs0 + 1], pre[:, W - 1:W])
                    hi = bass.AP(tensor=pre.tensor, offset=pre.offset + 2 * W - 1,
                                 ap=[pre.ap[0], [W, nrow - 1]])
                    lo = bass.AP(tensor=pre.tensor, offset=pre.offset + W - 1,
                                 ap=[pre.ap[0], [W, nrow - 1]])
                    nc.gpsimd.tensor_tensor(out=bins[:, cs0 + 1:cs0 + nrow],
                                            in0=hi, in1=lo, op=Alu.subtract)

                outw = opool.tile([128, C_SLAB], f32, tag="outw")
                nc.gpsimd.tensor_tensor(out=outw[:], in0=bins[:], in1=rct[:],
                                        op=Alu.mult)
                nc.sync.dma_start(out=out[win * RW:(win + 1) * RW, :],
                                  in_=outw[:])

    nc.compile()
    return nc


_PROG_CACHE = {}


def _get_program(pieces, repeat=1):
    key = (pieces, repeat)
    if key not in _PROG_CACHE:
        _PROG_CACHE[key] = build_program(pieces, repeat=repeat)
    return _PROG_CACHE[key]


# ---------------------------------------------------------------- entrypoint
def kernel(features: np.ndarray, rois: np.ndarray) -> np.ndarray:
    from concourse.bass_utils import run_bass_kernel_spmd

    features = np.asarray(features, dtype=np.float32)
    rois = np.asarray(rois, dtype=np.float32)

    order, pieces, hmask, mww, recip, mao, mbo, zz = _prep(rois)
    nc = _get_program(pieces)

    # [N, CS, H, W] -> [N, H, CS, W], bf16
    fbf = features.astype(_BF16)
    in_maps = []
    for k in range(NCORES):
        slab = fbf[:, k * C_SLAB:(k + 1) * C_SLAB].transpose(0, 2, 1, 3)
        in_maps.append({
            "fslab": np.ascontiguousarray(slab),
            "hmask": hmask,
            "mww": mww,
            "recip": recip,
        })

    res = run_bass_kernel_spmd(nc, in_maps, list(range(NCORES))).results

    result = np.empty((R, C_OUT, C_SLAB), dtype=np.float32)
    for k in range(NCORES):
        result[order, k, :] = res[k]["out"]
    return result.reshape(R, C_OUT, P, P)
